# revision 24
# baseline (speedup 1.0000x reference)
"""CosineCrossAttention Trainium2 kernel.

Math (per (b,t)):
    q = query @ Wq                      (N, D), heads head-major: d = h*48+dh
    k = kv @ Wk   (1, D);  v = kv @ Wv  (1, D)
    attn[n,h] = (q_h . k_h) / (|q_h||k_h|)
    out[n, dh*8+h] = attn[n,h] * v[h,dh];  out = out @ Wp + bp

Restructured:
    Kmat[d,h]  = k[d] * (d//48 == h)                 (D, H)
    Wqk        = Wq @ Kmat                           (D, H)
    attn_raw   = query @ Wqk        = q.k            (N, H)
    ss[n,h]    = sum_{d in head h} q[n,d]^2          via mask48 matmul on q^2
    attn       = attn_raw / (sqrt(ss) * |k_h|)
    v_perm     = kv @ Wv_perm   (Wv columns permuted so v_perm[d] = v[(d%8)*48+d//8])
    Wp_eff[h,:]= sum_d v_perm[d]*(d%8==h)*Wp[d,:]    (H, D)
    out        = attn @ Wp_eff + bp    (bias folded in as 9th contraction row)

Everything on-device runs in the transposed domain (D on partitions, N on free);
host passes query^T per (b,t). Sharding: data-parallel over B across 8 cores.

Perf structure (per 512-token group):
  S1  PE    : 9 matmuls  q^T chunks -> pq psum (3 banks)
  S2  Scalar: 3 squares  pq -> qsq sbuf (bf16)
  S3  PE    : 6 matmuls interleaved col-strips (0,0)/(0,32) -> par bank
              (attn_raw rows 0:8 || head-sumsq rows 32:40, concurrent tiles)
  S4  Scalar: sqrt(par[32:40]) -> nrm
  S5  Vector: reciprocal(nrm); att = par[0:8] * rnk * rcp -> att_sb (bf16)
  S6  PE    : 3 matmuls  out^T = [Wp_eff; bp]^T @ [att; 1]  (bias via matmul)
  S7  Scalar/Vector: psum -> osb sbuf (bf16)
  DMA : input qT per (t,half) on sync ring; output per t on scalar ring (bf16)
Stages S3..S5 lag one group, S6..S7 lag two groups (software pipeline).
"""

import sys

sys.path.insert(0, "/opt/trn_rl_repo")

from contextlib import ExitStack

import ml_dtypes
import numpy as np

import concourse.bass as bass
import concourse.tile as tile
from concourse import bacc, mybir
from concourse.masks import make_identity

F32 = mybir.dt.float32

B, T, N, D, H, Dh = 8, 8, 2048, 384, 8, 48
P = 128
CH = D // P  # 3 chunks of the D dims
NG = 512  # n-group (one PSUM bank of f32)

COMPUTE_DTYPE = "bf16"  # "bf16" | "f32r" | "f32"

_CDT = {
    "bf16": mybir.dt.bfloat16,
    "f32r": mybir.dt.float32r,
    "f32": mybir.dt.float32,
}
_NPDT = {
    "bf16": ml_dtypes.bfloat16,
    "f32r": np.float32,
    "f32": np.float32,
}


def build_nc(t_dim=T, n_dim=N, ng=NG, cdtype=None):
    cdtype = cdtype or COMPUTE_DTYPE
    CD = _CDT[cdtype]
    ngrp = n_dim // ng
    qsplit = 2 if ngrp >= 2 else 1
    gph = ngrp // qsplit  # groups per query-slice tile
    nh = gph * ng
    nc = bacc.Bacc("TRN2", target_bir_lowering=False, debug=False)

    qT = nc.dram_tensor("qT", [t_dim, D, n_dim], CD, kind="ExternalInput").ap()
    kvT = nc.dram_tensor("kvT", [D, t_dim], CD, kind="ExternalInput").ap()
    wq_d = nc.dram_tensor("Wq", [D, D], CD, kind="ExternalInput").ap()
    wqT_d = nc.dram_tensor("WqT", [D, D], CD, kind="ExternalInput").ap()
    wk_d = nc.dram_tensor("Wk", [D, D], CD, kind="ExternalInput").ap()
    wv_d = nc.dram_tensor("Wvp", [D, D], CD, kind="ExternalInput").ap()
    wp_d = nc.dram_tensor("Wp", [D, D], CD, kind="ExternalInput").ap()
    bp_d = nc.dram_tensor("bp", [D], F32, kind="ExternalInput").ap()
    m48_d = nc.dram_tensor("m48", [D, H], CD, kind="ExternalInput").ap()
    mv_d = nc.dram_tensor("mv", [D, H], CD, kind="ExternalInput").ap()
    outT = nc.dram_tensor("outT", [t_dim, D, n_dim], CD, kind="ExternalOutput").ap()

    with tile.TileContext(nc) as tc, ExitStack() as ctx:
        consts = ctx.enter_context(tc.tile_pool(name="consts", bufs=1))
        qpool = ctx.enter_context(tc.tile_pool(name="qpool", bufs=4))
        qsqp = ctx.enter_context(tc.tile_pool(name="qsqp", bufs=4))
        small = ctx.enter_context(tc.tile_pool(name="small", bufs=3))
        osbp = ctx.enter_context(tc.tile_pool(name="osbp", bufs=3))
        # PSUM budget: pq 2 + par2 2x2 + po 2 = 8 banks
        psA = ctx.enter_context(tc.tile_pool(name="psA", bufs=2, space="PSUM"))
        parp = ctx.enter_context(tc.tile_pool(name="parp", bufs=2, space="PSUM"))
        psC = ctx.enter_context(tc.tile_pool(name="psC", bufs=2, space="PSUM"))

        # ---------- constants (small, first in the DMA queue) ----------
        def load_w(dram, tag):
            sb = consts.tile([P, CH, D], CD, tag=tag)
            nc.sync.dma_start(sb, dram.rearrange("(c p) f -> p c f", p=P))
            return sb

        wq_sb = load_w(wq_d, "wq")
        wk_sb = load_w(wk_d, "wk")
        wv_sb = load_w(wv_d, "wv")
        wp_sb = load_w(wp_d, "wp")
        wqT_sb = load_w(wqT_d, "wqT")

        m48_sb = consts.tile([P, CH, H], CD, tag="m48")
        nc.sync.dma_start(m48_sb, m48_d.rearrange("(c p) h -> p c h", p=P))
        mv_sb = consts.tile([P, CH, H], CD, tag="mv")
        nc.sync.dma_start(mv_sb, mv_d.rearrange("(c p) h -> p c h", p=P))
        kvt_sb = consts.tile([P, CH, t_dim], CD, tag="kvt")
        nc.sync.dma_start(kvt_sb, kvT.rearrange("(c p) t -> p c t", p=P))

        # Wp_eff rows (slot-replicated below so contraction rows line up with
        # the per-quad att slot partitions)
        HI0 = 32 * (ngrp - 1) + H
        wpe = consts.tile([HI0, t_dim, D], CD, tag="wpe")
        bp_sb = consts.tile([P, CH], F32, tag="bp")
        nc.sync.dma_start(bp_sb, bp_d.rearrange("(c p) -> p c", p=P))

        # ---------- k/v projections for all t ----------
        ps_k = psA.tile([t_dim, D], F32, tag="pq")
        ps_v = psA.tile([t_dim, D], F32, tag="pq")
        for c in range(CH):
            nc.tensor.matmul(
                ps_k, kvt_sb[:, c, :], wk_sb[:, c, :],
                start=(c == 0), stop=(c == CH - 1),
            )
        for c in range(CH):
            nc.tensor.matmul(
                ps_v, kvt_sb[:, c, :], wv_sb[:, c, :],
                start=(c == 0), stop=(c == CH - 1),
            )

        k_sb = consts.tile([t_dim, D], F32, tag="k_sb")
        nc.scalar.copy(k_sb, ps_k)
        vsb = consts.tile([t_dim, D], F32, tag="vsb")
        nc.scalar.copy(vsb, ps_v)

        # transpose k, v -> (D-part, t); cast to compute dtype on copyback
        kT = consts.tile([P, CH, t_dim], CD, tag="kT")
        vT = consts.tile([P, CH, t_dim], CD, tag="vT")
        idt = consts.tile([t_dim, t_dim], F32, tag="idt")
        make_identity(nc, idt)
        for c in range(CH):
            pt = psC.tile([P, t_dim], F32, tag="po")
            nc.tensor.transpose(pt, k_sb[:, c * P : (c + 1) * P], idt)
            nc.vector.tensor_copy(kT[:, c, :], pt)
            pt2 = psC.tile([P, t_dim], F32, tag="po")
            nc.tensor.transpose(pt2, vsb[:, c * P : (c + 1) * P], idt)
            nc.vector.tensor_copy(vT[:, c, :], pt2)

        # per-head k norms: rnkT[h, t] = 1/|k_h|(t)
        ksqT = consts.tile([P, CH, t_dim], CD, tag="ksqT")
        nc.scalar.square(ksqT, kT)
        psk2 = psA.tile([40, ng], F32, tag="pq")
        for c in range(CH):
            nc.tensor.matmul(
                psk2[0:H, 0:t_dim], m48_sb[:, c, :], ksqT[:, c, :],
                start=(c == 0), stop=(c == CH - 1),
            )
        rnkT = consts.tile([H, t_dim], F32, tag="rnkT")
        nc.scalar.sqrt(rnkT, psk2[0:H, 0:t_dim])
        nc.vector.reciprocal(rnkT, rnkT)

        # Kmat[d, t, h] = kT[d, t] * m48[d, h];  Vsel[d, t, h] = vT[d, t] * mv[d, h]
        kmat = consts.tile([P, CH, t_dim, H], CD, tag="kmat")
        nc.vector.tensor_tensor(
            kmat,
            kT[:, :, :, None].to_broadcast((P, CH, t_dim, H)),
            m48_sb[:, :, None, :].to_broadcast((P, CH, t_dim, H)),
            op=mybir.AluOpType.mult,
        )
        vsel = consts.tile([P, CH, t_dim, H], CD, tag="vsel")
        nc.vector.tensor_tensor(
            vsel,
            vT[:, :, :, None].to_broadcast((P, CH, t_dim, H)),
            mv_sb[:, :, None, :].to_broadcast((P, CH, t_dim, H)),
            op=mybir.AluOpType.mult,
        )

        # Wqk[d_in, t, h] = sum_dmid Wq[d_in, dmid] Kmat[dmid, t, h]
        wqk = consts.tile([P, CH, t_dim, H], CD, tag="wqk")
        for ci in range(CH):
            pw = psA.tile([P, ng], F32, tag="pq")
            for cm in range(CH):
                nc.tensor.matmul(
                    pw[:, 0 : t_dim * H],
                    wqT_sb[:, cm, ci * P : (ci + 1) * P],
                    kmat[:, cm, :, :],
                    start=(cm == 0), stop=(cm == CH - 1),
                )
            nc.scalar.copy(
                wqk[:, ci], pw[:, 0 : t_dim * H].rearrange("p (t h) -> p t h", h=H)
            )

        # Wp_eff rows: wpe[h, t, d_out] = sum_d Vsel[d, t, h] * Wp[d, d_out]
        for t in range(t_dim):
            pe_t = psC.tile([P, ng], F32, tag="po")
            for c in range(CH):
                nc.tensor.matmul(
                    pe_t[0:H, 0:D], vsel[:, c, t, :], wp_sb[:, c, :],
                    start=(c == 0), stop=(c == CH - 1),
                )
            nc.scalar.copy(wpe[0:H, t, :], pe_t[0:H, 0:D])

        # att staging: per quad (= one t, ngrp groups), group l's attn values
        # live at partitions 32l..32l+8 so the whole quad's normalization runs
        # as single full-width ops.
        HI = 32 * (ngrp - 1) + H
        att_all = consts.tile([HI, 2, ng], CD, tag="att")

        # rnkq: rnkT replicated at each 32-partition slot (stt scalar operand);
        # non-slot rows are read by the full-span stt, so initialize them.
        rnkq = consts.tile([P, t_dim], F32, tag="rnkq")
        nc.gpsimd.memset(rnkq, 1.0)
        for l in range(ngrp):
            nc.scalar.copy(rnkq[32 * l : 32 * l + H, :], rnkT)

        # wpe replicated at each slot (matmul contraction rows must line up
        # with the att slot partitions)
        for l in range(1, ngrp):
            nc.scalar.copy(wpe[32 * l : 32 * l + H, :, :], wpe[0:H, :, :])

        # ---------- main loop (software-pipelined) ----------
        # per iteration g: S1/S2(g); attn(g-1) || ss(g-2) staggered-paired on
        # disjoint col strips; quad chain (sqrt/recip/stt) when ss finishes a
        # quad; one S6/S7 group per iteration from the ready queue.
        s1rec = {}
        par2_of = {}
        s6q = []
        s6flip = [0]

        def emit_pair(A, S):
            if A is not None:
                tA, qtA, qslA = s1rec[A][0], s1rec[A][1], s1rec[A][2]
                a = A % ngrp
                parA = par2_of[A // ngrp]
            if S is not None:
                qsqS = s1rec[S][3]
                s = S % ngrp
                parS = par2_of[S // ngrp]
            for c in range(CH):
                if A is not None:
                    nc.tensor.matmul(
                        parA[32 * a : 32 * a + H, 0, :],
                        wqk[:, c, tA, :], qtA[:, c, qslA],
                        start=(c == 0), stop=(c == CH - 1),
                        tile_position=(0, 32 * a), skip_group_check=True,
                    )
                if S is not None:
                    nc.tensor.matmul(
                        parS[32 * s : 32 * s + H, 1, :],
                        m48_sb[:, c, :], qsqS[:, c, :],
                        start=(c == 0), stop=(c == CH - 1),
                        tile_position=(0, 32 * s), skip_group_check=True,
                    )
            if S is not None and S % ngrp == ngrp - 1:
                # quad complete: normalization chain over all ngrp groups at once
                q = S // ngrp
                tq = s1rec[S][0]
                par2 = par2_of.pop(q)
                nrm2 = small.tile([P, ng], F32, tag="nrm2")
                nc.scalar.sqrt(nrm2, par2[:, 1, :])
                rcp2 = small.tile([P, ng], F32, tag="rcp2")
                nc.vector.reciprocal_approx_fast(rcp2, nrm2)
                nc.vector.scalar_tensor_tensor(
                    att_all[0:HI, q % 2, :], par2[0:HI, 0, :],
                    rnkq[0:HI, tq : tq + 1], rcp2[0:HI, :],
                    op0=mybir.AluOpType.mult, op1=mybir.AluOpType.mult,
                )
                for j in range(q * ngrp, (q + 1) * ngrp):
                    rj = s1rec.pop(j)
                    s6q.append((rj[0], rj[4], j % ngrp, q % 2, rj[5], rj[6]))

        def emit_s6s7(t, sl, l, qb, osb, last):
            for co in range(CH):
                po = psC.tile([P, ng], F32, tag="po")
                nc.tensor.matmul(
                    po, wpe[32 * l : 32 * l + H, t, co * P : (co + 1) * P],
                    att_all[32 * l : 32 * l + H, qb, :],
                    start=True, stop=True, tile_position=(32 * l, 0),
                )
                # evacuate + bias; alternate the first chunk between engines
                if co == 0 and s6flip[0] % 2 == 0:
                    nc.scalar.activation(
                        osb[:, co, sl], po,
                        mybir.ActivationFunctionType.Identity,
                        bias=bp_sb[:, co : co + 1], scale=1.0,
                    )
                else:
                    nc.vector.tensor_tensor(
                        osb[:, co, sl], po,
                        bp_sb[:, co : co + 1].to_broadcast((P, ng)),
                        op=mybir.AluOpType.add,
                    )
            s6flip[0] += 1
            if last:
                nc.scalar.dma_start(
                    outT[t].rearrange("(c p) n -> p c n", p=P), osb
                )

        def iterate(g, t, qt, qsl, sl, osb, last):
            if g % ngrp == 0:
                par2 = parp.tile([P, 2, ng], F32, tag="par2")
                nc.vector.memset(par2, 1.0)
                par2_of[g // ngrp] = par2
            qsq = qsqp.tile([P, CH, ng], CD, tag="qsq")
            for co in range(CH):
                pq = psA.tile([P, ng], F32, tag="pq")
                for c in range(CH):
                    nc.tensor.matmul(
                        pq, wq_sb[:, c, co * P : (co + 1) * P], qt[:, c, qsl],
                        start=(c == 0), stop=(c == CH - 1),
                    )
                nc.scalar.square(qsq[:, co, :], pq)
            s1rec[g] = (t, qt, qsl, qsq, sl, osb, last)
            emit_pair(g - 1 if g >= 1 else None, g - 2 if g >= 2 else None)
            if s6q:
                emit_s6s7(*s6q.pop(0))

        g = 0
        for t in range(t_dim):
            osb = osbp.tile([P, CH, n_dim], CD, tag="osb")
            for hf in range(qsplit):
                qt = qpool.tile([P, CH, nh], CD, tag="qt")
                nc.sync.dma_start(
                    qt,
                    qT[t].rearrange("(c p) n -> p c n", p=P)[
                        :, :, hf * nh : (hf + 1) * nh
                    ],
                )
                for gl in range(gph):
                    qsl = slice(gl * ng, (gl + 1) * ng)
                    sl = slice(hf * nh + gl * ng, hf * nh + (gl + 1) * ng)
                    last = (hf == qsplit - 1) and (gl == gph - 1)
                    iterate(g, t, qt, qsl, sl, osb, last)
                    g += 1
        # drain: two virtual iterations flush attn/ss, then the s6 queue
        emit_pair(g - 1, g - 2 if g >= 2 else None)
        emit_pair(None, g - 1)
        while s6q:
            emit_s6s7(*s6q.pop(0))

    nc.compile()
    return nc


_CACHE = {}


def _get_nc(t_dim=T, n_dim=N):
    key = (t_dim, n_dim, COMPUTE_DTYPE)
    if key not in _CACHE:
        _CACHE[key] = build_nc(t_dim, n_dim)
    return _CACHE[key]


def _host_prep(query, kv, Wq, Wk, Wv, Wp, bp):
    ndt = _NPDT[COMPUTE_DTYPE]
    query = np.asarray(query, dtype=np.float32)
    kv = np.asarray(kv, dtype=np.float32)
    Wq = np.ascontiguousarray(np.asarray(Wq, dtype=np.float32).astype(ndt))
    WqT = np.ascontiguousarray(Wq.T)
    Wk = np.ascontiguousarray(np.asarray(Wk, dtype=np.float32).astype(ndt))
    Wv = np.asarray(Wv, dtype=np.float32)
    Wp = np.ascontiguousarray(np.asarray(Wp, dtype=np.float32).astype(ndt))
    bp = np.asarray(bp, dtype=np.float32)

    b_dim, t_dim, n_dim, d = query.shape
    dh = d // H
    # Wv with columns permuted: v_perm[d] = v[(d%H)*dh + d//H]
    perm = (np.arange(d) % H) * dh + np.arange(d) // H
    Wvp = np.ascontiguousarray(Wv[:, perm].astype(ndt))
    dd = np.arange(d)
    hh = np.arange(H)
    m48 = (dd[:, None] // dh == hh[None, :]).astype(ndt)
    mv = (dd[:, None] % H == hh[None, :]).astype(ndt)
    bp = np.ascontiguousarray(bp)

    in_maps = []
    for b in range(b_dim):
        in_maps.append(
            {
                "qT": np.ascontiguousarray(query[b].transpose(0, 2, 1).astype(ndt)),
                "kvT": np.ascontiguousarray(kv[b, :, 0, :].T.astype(ndt)),
                "Wq": Wq,
                "WqT": WqT,
                "Wk": Wk,
                "Wvp": Wvp,
                "Wp": Wp,
                "bp": bp,
                "m48": m48,
                "mv": mv,
            }
        )
    return in_maps, (b_dim, t_dim, n_dim, d)


def _gather(results, shape):
    b_dim, t_dim, n_dim, d = shape
    out = np.empty((b_dim, t_dim, n_dim, d), dtype=np.float32)
    for b in range(b_dim):
        out[b] = results[b]["outT"].transpose(0, 2, 1).astype(np.float32)
    return out


def kernel(query, kv, Wq, Wk, Wv, Wp, bp):
    from concourse.bass_utils import run_bass_kernel_spmd

    in_maps, shape = _host_prep(query, kv, Wq, Wk, Wv, Wp, bp)
    nc = _get_nc(shape[1], shape[2])
    res = run_bass_kernel_spmd(nc, in_maps, core_ids=list(range(len(in_maps))))
    return _gather(res.results, shape)


def _install_ntff_hook():
    """The agent image's antenv lacks axon_hooks; synthesize it so
    run_bass_kernel_spmd(trace=True) can capture NTFF profiles."""
    import types

    if "antenv.axon_hooks" in sys.modules:
        return
    sys.path.insert(0, "/root/.axon_site")
    from trn_agent_boot.trn_boot import _ntff_profile_via_ctypes

    hook = _ntff_profile_via_ctypes("/opt/axon/libaxon_pjrt.so")
    mod = types.ModuleType("antenv.axon_hooks")
    mod.get_axon_ntff_profile_hook = lambda: hook
    mod.set_axon_ntff_profile_hook = lambda h: None
    sys.modules["antenv.axon_hooks"] = mod


def kernel_traced(query, kv, Wq, Wk, Wv, Wp, bp):
    """Like kernel() but captures an NTFF profile; returns (out, results)."""
    from concourse.bass_utils import run_bass_kernel_spmd

    _install_ntff_hook()
    in_maps, shape = _host_prep(query, kv, Wq, Wk, Wv, Wp, bp)
    nc = _get_nc(shape[1], shape[2])
    res = run_bass_kernel_spmd(
        nc, in_maps, core_ids=list(range(len(in_maps))), trace=True
    )
    return _gather(res.results, shape), res
